# revision 34
# baseline (speedup 1.0000x reference)
"""CPC (contrastive predictive coding) loss on 8 Trainium2 NeuronCores.

Problem: loss = mean over (t, k, i) of cross_entropy(scores[t,k,i,:], i) with
scores[t,k,i,j] = <c_proj[i,t], z[j,t+k]> / TEMP,  c_proj = c_seq @ W + b,
t in [0, Tm), k in [1, H], i,j in [0, B).

Distribution: sequence-parallel over anchor time t.  Every core runs an
identical program over TSLOT=14 anchor slots (7 "pair tiles" of 2 consecutive
anchors each); cores with fewer real anchors carry zero-padded slots whose
contributions are removed by per-core validity masks.  Each core returns a
(128,1) partial-sum vector plus the per-group softmax sums; the host adds the
log terms (cheap there) and divides by the term count.

Per-core device pipeline (all matmuls bf16 inputs, fp32 accumulation):
  1. Contiguous DMA loads of pre-transposed (host-side) z^T, c^T, W.
  2. c_projT = (W-chunk as lhsT) @ c^T via PE; bias added during the
     PSUM->SBUF copy on the scalar engine, cast to bf16.
  3. Per pair tile (anchors t,t+1): one (128 x 31*64) PSUM scores tile via 16
     matmuls (lhsT = c_projT columns, rhs = z^T columns of the 31-shift
     window).
  4. Softmax statistics with the broadcast-subtract moved OFF the DVE (the
     bottleneck engine -- reductions are always 1x):
       - grouped reduce_max (DVE, negated, bf16 out)
       - the per-group max is subtracted on the PE by an accumulating matmul:
         lhsT = identity, rhs = negmax broadcast over j, start=False
       - exp with scale=1/TEMP straight out of PSUM on ACT
       - grouped sum over j (DVE); log happens on the host
  5. The positive terms via banded Gram matmuls, 4 batch elements per matmul
     (128-column weights -> fast weight load), diag+band masked accumulation.

The For_i benchmark loop runs TWO iterations per trip with double-buffered
inputs: each iteration's DMA loads are issued while the previous iteration
computes, so the (slow, ~40us for 4.3MB/core) HBM loads hide under compute.
"""

import numpy as np
import ml_dtypes

B, T, D = 64, 128, 512
H = 30
TEMP = 0.07
NCORE = 8
TSLOT = 14            # padded anchor slots per core -> 7 pair tiles
NPAIR = TSLOT // 2
TS = TSLOT - 1 + H    # 43 z timesteps per core (slab + horizon halo)
G = H + 1             # 31 shift groups per pair tile
KCH = D // 128        # 4 contraction chunks
TM = T - H            # 98 real anchors
NBATCH = B // 4       # 16 pos-matmul batches (4 i's per matmul)

_REAL = [13, 13, 12, 12, 12, 12, 12, 12]
_T0 = [0, 13, 26, 38, 50, 62, 74, 86]

_CACHE = {}


def _build_program(loop_n=None, variant="full"):
    import concourse.bass as bass
    import concourse.bacc as bacc
    import concourse.tile as tile
    import concourse.mybir as mybir
    from contextlib import ExitStack

    # variant may carry loop-mode suffixes: e.g. "full+stg" (staggered-reset
    # back edge), "full+hint" (branch prefetch hints)
    parts = variant.split("+")
    variant, loop_flags = parts[0], set(parts[1:])

    dt = mybir.dt
    AF = mybir.ActivationFunctionType
    ALU = mybir.AluOpType
    AX = mybir.AxisListType

    nc = bacc.Bacc("TRN2", debug=False, target_bir_lowering=False,
                   num_devices=NCORE)

    # all per-iteration inputs ride in ONE contiguous blob per core:
    # per k-chunk (4 chunks of 128 d-partitions): [zT | cT | W | b] bytes
    ZB = TS * B * 2            # 5504 z bytes per partition row
    CB = B * TSLOT * 2         # 1792 c bytes
    WB = D * 2                 # 1024 w bytes
    BB = 4                     # 4 bias bytes (fp32)
    CHB = ZB + CB + WB + BB    # 8324 bytes per chunk
    blob_d = nc.dram_tensor("blob", [128, KCH * CHB], dt.uint8,
                            kind="ExternalInput").ap()
    vm_d = nc.dram_tensor("vm", [128, NPAIR * G], dt.bfloat16, kind="ExternalInput").ap()
    bd_d = nc.dram_tensor("band4", [128, 4 * TS], dt.bfloat16, kind="ExternalInput").ap()
    id_d = nc.dram_tensor("id_bf", [128, 128], dt.bfloat16, kind="ExternalInput").ap()
    out_d = nc.dram_tensor("partial", [128, 1], dt.float32, kind="ExternalOutput").ap()
    s_d = nc.dram_tensor("s_out", [128, NPAIR * G], dt.float32,
                         kind="ExternalOutput").ap()

    NROW = B * TSLOT          # 896 c rows
    GB = G * B                # 1984 columns of a pair tile
    NACC = 12                 # accumulator columns: max, 8x pos, spare
    inv_t = 1.0 / TEMP

    with tile.TileContext(nc) as tc, ExitStack() as ctx:
        con = ctx.enter_context(tc.tile_pool(name="con", bufs=1))
        wrk = ctx.enter_context(tc.tile_pool(name="wrk", bufs=4))

        # ---- program constants: loaded once, never reloaded in the loop ----
        vm_sb = con.tile([128, NPAIR * G], dt.bfloat16, tag="vm", name="vm_sb")
        nc.sync.dma_start(vm_sb[:], vm_d)
        bd_sb = con.tile([128, 4 * TS], dt.bfloat16, tag="bd", name="bd_sb")
        nc.sync.dma_start(bd_sb[:], bd_d)
        id_sb = con.tile([128, 128], dt.bfloat16, tag="id", name="id_sb")
        nc.sync.dma_start(id_sb[:], id_d)

        def _alloc_inputs(sfx):
            t = {}
            blob = con.tile([128, KCH * CHB], dt.uint8, tag=f"blob{sfx}",
                            name=f"blob{sfx}")
            t["blob"] = blob
            t["zt"] = [blob[:, k * CHB:k * CHB + ZB].bitcast(dt.bfloat16)
                       for k in range(KCH)]
            t["ct"] = [blob[:, k * CHB + ZB:k * CHB + ZB + CB].bitcast(dt.bfloat16)
                       for k in range(KCH)]
            t["w"] = [blob[:, k * CHB + ZB + CB:k * CHB + ZB + CB + WB]
                      .bitcast(dt.bfloat16) for k in range(KCH)]
            t["b"] = [blob[:, k * CHB + ZB + CB + WB:(k + 1) * CHB]
                      .bitcast(dt.float32) for k in range(KCH)]
            # c_projT at a 32-column-per-batch-element pitch for the pos
            # matmuls (contiguous 128-col lhsT slices -> fast weight load);
            # pad columns are zeroed once in the prologue and never rewritten
            t["cp32"] = [con.tile([128, B * 32], dt.bfloat16, tag=f"cp{k}{sfx}",
                                  name=f"cp32_{k}{sfx}") for k in range(KCH)]
            for k in range(KCH):
                nc.vector.memset(t["cp32"][k][:], 0.0)
            return t

        def _loads(t):
            # one contiguous DMA for everything this iteration reads
            if variant == "dmahalf":
                nc.gpsimd.dma_start(t["blob"][:, 0:2 * CHB], blob_d[:, 0:2 * CHB])
                return
            if "spq" in loop_flags:
                h = KCH * CHB // 2
                nc.sync.dma_start(t["blob"][:, 0:h], blob_d[:, 0:h])
                nc.scalar.dma_start(t["blob"][:, h:], blob_d[:, h:])
            else:
                nc.gpsimd.dma_start(t["blob"][:], blob_d)

        def _compute(t, sfx):
            b_sb, w_sb, ct_sb, zt_sb = t["b"], t["w"], t["ct"], t["zt"]
            cp_sb = t["cp32"]

            acc = con.tile([128, NACC], dt.float32, tag="acc", name=f"acc{sfx}")
            nc.vector.memset(acc[:], 0.0)
            if variant in ("dmaonly", "dmahalf"):
                for k in range(KCH if variant == "dmaonly" else 2):
                    nc.vector.tensor_reduce(acc[:, 0:1], zt_sb[k][:, 0:64],
                                            axis=AX.X, op=ALU.add)
                for k in range(KCH):
                    nc.vector.tensor_reduce(acc[:, 1:2], ct_sb[k][:, 0:64],
                                            axis=AX.X, op=ALU.add)
                    nc.vector.tensor_reduce(acc[:, 2:3], w_sb[k][:, 0:64],
                                            axis=AX.X, op=ALU.add)
            s_all = con.tile([128, NPAIR * G], dt.float32, tag="sall",
                             name=f"s_all{sfx}")

            # ---------------- c_projT (bf16, two layouts) ------------
            # cp32: (d_out, (i, slot-pad-32)) -> pos matmul weights
            # cq:   (d_out, (t, i))           -> pair-tile matmul weights
            cq_sb = []
            with tc.tile_pool(name=f"pcp{sfx}", bufs=2, space="PSUM") as pcp:
                for m in range(0 if variant in ("dmaonly", "dmahalf") else KCH):
                    psc = pcp.tile([128, NROW], dt.float32, tag="psc", name="psc")
                    for (n0, nn) in ((0, 512), (512, NROW - 512)):
                        for k in range(KCH):
                            nc.tensor.matmul(
                                psc[:, n0:n0 + nn],
                                w_sb[k][:, m * 128:(m + 1) * 128],
                                ct_sb[k][:, n0:n0 + nn],
                                start=(k == 0), stop=(k == KCH - 1),
                            )
                    nc.scalar.activation(
                        cp_sb[m][:].rearrange("p (i s) -> p i s", s=32)[:, :, 0:TSLOT],
                        psc[:].rearrange("p (i t) -> p i t", t=TSLOT),
                        AF.Identity, bias=b_sb[m])
                    cq = con.tile([128, NROW], dt.bfloat16, tag=f"cq{m}",
                                  name=f"cq_sb{m}{sfx}")
                    nc.scalar.activation(
                        cq[:], psc[:].rearrange("p (i t) -> p t i", t=TSLOT),
                        AF.Identity, bias=b_sb[m])
                    cq_sb.append(cq)

            # ---------------- 7 pair tiles ----------------
            # Software-pipelined emission: tile p's scores matmuls are emitted
            # before tile p-1's subtract/exp/sum so the PE never sits behind
            # the DVE max in its own instruction stream.
            NCH = ((0, 8), (8, 8), (16, 8), (24, G - 24))
            # per-tile bf16 negmax collected here; one masked accumulation of
            # the (bf16-rounded, i.e. exactly-as-subtracted) max at the end
            negmax_all = con.tile([128, NPAIR * G], dt.bfloat16, tag="nmx",
                                  name=f"negmax_all{sfx}")
            with tc.tile_pool(name=f"pps{sfx}", bufs=2, space="PSUM") as pps:
                ps_q = []

                def _emit_scores(p):
                    ps = pps.tile([128, GB], dt.float32, tag="ps", name="ps")
                    for (g0, gn) in NCH:
                        for k in range(KCH):
                            lhsT = cq_sb[k][:, 2 * p * B:(2 * p + 2) * B]
                            rhs = zt_sb[k][:, (2 * p + g0) * B:(2 * p + g0 + gn) * B]
                            nc.tensor.matmul(
                                ps[:, g0 * B:(g0 + gn) * B], lhsT, rhs,
                                start=(k == 0), stop=(k == KCH - 1),
                            )
                    return ps

                def _emit_max(p, ps):
                    # negated per-group max, rounded to bf16 (the exact value
                    # the PE will subtract and the value re-added at the end)
                    ps3 = ps[:].rearrange("p (g j) -> p g j", j=B)
                    negmax = negmax_all[:, p * G:(p + 1) * G]
                    nc.vector.tensor_reduce(negmax, ps3, axis=AX.X, op=ALU.max,
                                            negate=True)
                    return negmax

                def _emit_softmax(p, ps, negmax):
                    # subtract the group max on the PE:
                    #   ps[m, (g,j)] += negmax[m, g]   (lhsT = identity)
                    for (g0, gn) in NCH:
                        nc.tensor.matmul(
                            ps[:, g0 * B:(g0 + gn) * B],
                            id_sb[:],
                            negmax[:, g0:g0 + gn].broadcast_to((128, gn, B)),
                            start=False, stop=True, skip_group_check=True,
                        )
                    # exp((x - max)/TEMP) straight out of PSUM, bf16 out
                    esb = wrk.tile([128, GB], dt.bfloat16, tag="esb", name="esb")
                    nc.scalar.activation(esb[:], ps[:], AF.Exp, scale=inv_t)
                    # grouped sum over j: one bf16 2x pairwise add halves the
                    # stream the (always-1x) reduce has to chew through
                    e3 = esb[:].rearrange("p (g j) -> p g j", j=B)
                    eh = wrk.tile([128, G * B // 2], dt.bfloat16, tag="eh",
                                  name="eh")
                    eh3 = eh[:].rearrange("p (g j) -> p g j", j=B // 2)
                    nc.vector.tensor_tensor(eh3, e3[:, :, 0:B // 2],
                                            e3[:, :, B // 2:B], op=ALU.add)
                    s_t = s_all[:, p * G:(p + 1) * G]
                    nc.vector.tensor_reduce(s_t, eh3, axis=AX.X, op=ALU.add)

                for p in range(0 if variant in ("dmaonly", "dmahalf") else NPAIR):
                    ps = _emit_scores(p)
                    if variant == "noce":
                        junkc = wrk.tile([128, 1], dt.float32, tag="junkc",
                                         name="junkc")
                        nc.vector.tensor_reduce(junkc[:], ps[:, 0:B],
                                                axis=AX.X, op=ALU.add)
                        continue
                    nm = _emit_max(p, ps)
                    ps_q.append((p, ps, nm))
                    if len(ps_q) >= 2:
                        _emit_softmax(*ps_q.pop(0))
                while ps_q:
                    _emit_softmax(*ps_q.pop(0))

            if variant in ("full", "nopos"):
                # masked accumulation of sum over valid groups of (+max/TEMP):
                # the lse reconstruction term for the shifted softmax
                junk2 = wrk.tile([128, NPAIR * G], dt.float32, tag="junk2",
                                 name="junk2")
                nc.vector.scalar_tensor_tensor(
                    junk2[:], negmax_all[:], -inv_t, vm_sb[:], op0=ALU.mult,
                    op1=ALU.mult, accum_out=acc[:, 1:2])

            # ------------- positive terms: banded Gram matmuls -------------
            # One matmul covers 4 batch elements with a FULL 128-column
            # stationary operand (fast weight load): lhsT = cp columns
            # [i4*14 .. i4*14+32) x4 (overlapping strided windows), rhs = the
            # same 4 elements' z columns (4*43, strided).  Output partition
            # p = 32*i4 + slot; the diag+band mask bd_sb zeroes cross-element
            # blocks, pad rows, and out-of-band (slot, s) pairs.
            # 2 groups pack per PSUM bank at a 256-column pitch.
            with tc.tile_pool(name=f"ppo{sfx}", bufs=3, space="PSUM") as ppo:
                for b0 in range(0, NBATCH if variant == "full" else 0, 2):
                    pp = ppo.tile([128, 512], dt.float32, tag="pp", name="pp")
                    for u in range(2):
                        bi = b0 + u
                        for k in range(KCH):
                            lhsT = cp_sb[k][:, 4 * bi * 32:4 * bi * 32 + 128]
                            rhs = zt_sb[k].rearrange(
                                "p (s i) -> p i s", i=B)[:, 4 * bi:4 * bi + 4, :]
                            nc.tensor.matmul(
                                pp[:, u * 256:u * 256 + 4 * TS], lhsT, rhs,
                                start=(k == 0), stop=(k == KCH - 1),
                            )
                    junk3 = wrk.tile([128, 2 * 4 * TS], dt.float32, tag="junk3",
                                     name="junk3")
                    nc.vector.scalar_tensor_tensor(
                        junk3[:].rearrange("p (u c) -> p u c", c=4 * TS),
                        pp[:].rearrange("p (u c) -> p u c", c=256)[:, :, 0:4 * TS],
                        -inv_t,
                        bd_sb[:].rearrange("p c -> p () c").broadcast_to(
                            (128, 2, 4 * TS)),
                        op0=ALU.mult, op1=ALU.mult,
                        accum_out=acc[:, 3 + b0 // 2:4 + b0 // 2])

            if variant in ("dmaonly", "dmahalf", "noce"):
                part0 = con.tile([128, 1], dt.float32, tag="part", name=f"part0{sfx}")
                nc.vector.tensor_reduce(part0[:], acc[:], axis=AX.X, op=ALU.add)
                nc.sync.dma_start(out_d, part0[:])
                return
            # the log of the per-group softmax sums happens on the host (it
            # is tiny there and saves an ACT table-set swap per iteration)
            nc.sync.dma_start(s_d, s_all[:])
            part = con.tile([128, 1], dt.float32, tag="part", name=f"part{sfx}")
            nc.vector.tensor_reduce(part[:], acc[:], axis=AX.X, op=ALU.add)
            nc.sync.dma_start(out_d, part[:])

        if loop_n:
            assert loop_n % 2 == 0, "loop_n must be even (2 iterations/trip)"
            kw = {}
            if "stg" in loop_flags:
                kw["staggered_reset"] = True
            if "hint" in loop_flags:
                kw["hint_engines"] = (mybir.EngineType.PE,
                                      mybir.EngineType.Activation,
                                      mybir.EngineType.DVE,
                                      mybir.EngineType.SP,
                                      mybir.EngineType.Pool)
            tA = _alloc_inputs("a")
            if "noload" in loop_flags:
                # loads only in the prologue: measures pure compute
                _loads(tA)
                with tc.For_i(0, loop_n // 2, 1, **kw):
                    _compute(tA, "a")
                    _compute(tA, "b")
            else:
                tB = _alloc_inputs("b")
                _loads(tA)
                with tc.For_i(0, loop_n // 2, 1, **kw):
                    _loads(tB)        # prefetch B while computing A
                    _compute(tA, "a")
                    _loads(tA)        # prefetch next trip's A while computing B
                    _compute(tB, "b")
        else:
            tA = _alloc_inputs("a")
            _loads(tA)
            _compute(tA, "a")

    nc.compile()
    return nc


def get_program(loop_n=None, variant="full"):
    key = ("nc", loop_n, variant)
    if key not in _CACHE:
        _CACHE[key] = _build_program(loop_n, variant)
    return _CACHE[key]


def make_core_inputs(m, z, c, W, b):
    """Host-side sharding + bf16 cast for core m."""
    bf = ml_dtypes.bfloat16
    t0, nreal = _T0[m], _REAL[m]

    # device-side layouts: zT (D, (s, i)), cT (D, (i, t)) -- transposed on
    # the host so the device does plain contiguous DMA loads (no xbar)
    s_lo = t0 + 1
    n_avail = min(TS, T - s_lo)
    zslab = np.zeros((D, TS, B), dtype=bf)
    zslab[:, :n_avail] = z[:, s_lo:s_lo + n_avail].astype(bf).transpose(2, 1, 0)
    zslab = zslab.reshape(D, TS * B)

    cslab = np.zeros((D, B, TSLOT), dtype=bf)
    cslab[:, :, :nreal] = c[:, t0:t0 + nreal].astype(bf).transpose(2, 0, 1)
    cslab = cslab.reshape(D, B * TSLOT)

    # pair-tile validity: partition p = half*64 + i, half anchored at t+half
    p_idx = np.arange(128)
    g_idx = np.arange(G)
    th = p_idx[:, None, None] // B                     # (128,1,1)
    pp = np.arange(NPAIR)[None, :, None]               # (1,7,1)
    gg = g_idx[None, None, :]                          # (1,1,31)
    slot = 2 * pp + th
    gvalid = np.where(th == 0, gg <= H - 1, (gg >= 1) & (gg <= H))
    vm = ((slot < nreal) & gvalid).astype(np.float32).reshape(128, NPAIR * G)

    # pos diag+band mask: partition p = 32*i4 + slot, column = (i', s);
    # valid iff i' == i4, slot is a real anchor, and s in [slot, slot+H)
    i4 = (p_idx // 32)[:, None, None]                  # (128,1,1)
    slot2 = (p_idx % 32)[:, None, None]                # (128,1,1)
    ii = np.arange(4)[None, :, None]                   # (1,4,1)
    si = np.arange(TS)[None, None, :]                  # (1,1,43)
    band4 = ((ii == i4) & (slot2 < nreal) & (si >= slot2)
             & (si < slot2 + H)).astype(np.float32).reshape(128, 4 * TS)

    # pack all per-iteration inputs into one [128, KCH*CHB] byte blob:
    # per k-chunk row p: [zT row | cT row | W row | bias value]
    wbf = W.astype(bf)
    bf32 = b.astype(np.float32)
    CHB = TS * B * 2 + B * TSLOT * 2 + D * 2 + 4
    blob = np.empty((KCH, 128, CHB), dtype=np.uint8)
    for k in range(KCH):
        r = slice(k * 128, (k + 1) * 128)
        o = 0
        for arr in (zslab[r], cslab[r], wbf[r]):
            ab = arr.view(np.uint8).reshape(128, -1)
            blob[k, :, o:o + ab.shape[1]] = ab
            o += ab.shape[1]
        blob[k, :, o:o + 4] = bf32[r, None].view(np.uint8).reshape(128, 4)
    blob = blob.transpose(1, 0, 2).reshape(128, KCH * CHB)

    return {
        "blob": blob,
        "vm": vm.astype(bf),
        "band4": band4.astype(bf),
        "id_bf": np.eye(128, dtype=bf),
    }


def reduce_results(results, in_maps):
    """Host-side finish: sum per-core partials + masked log of the softmax
    sums (the device ships S per group; log is cheap here)."""
    tot = 0.0
    for r, im in zip(results, in_maps):
        tot += float(r["partial"].astype(np.float64).sum())
        s = r["s_out"].astype(np.float64)
        vm = im["vm"].astype(np.float64)
        tot += float((np.log(np.maximum(s, 1e-300)) * vm).sum())
    return np.float32(tot / (TM * H * B))


def kernel(z_seq, c_seq, W_cpc, b_cpc):
    z = np.asarray(z_seq, dtype=np.float32)
    c = np.asarray(c_seq, dtype=np.float32)
    W = np.asarray(W_cpc, dtype=np.float32)
    b = np.asarray(b_cpc, dtype=np.float32)

    nc = get_program()
    in_maps = [make_core_inputs(m, z, c, W, b) for m in range(NCORE)]

    from concourse.bass_utils import run_bass_kernel_spmd
    res = run_bass_kernel_spmd(nc, in_maps, core_ids=list(range(NCORE)))

    return reduce_results(res.results, in_maps)


if __name__ == "__main__":
    rng = np.random.default_rng(0)
    out = kernel(
        rng.standard_normal((B, T, D), dtype=np.float32),
        rng.standard_normal((B, T, D), dtype=np.float32),
        (rng.standard_normal((D, D)) / np.sqrt(D)).astype(np.float32),
        (rng.standard_normal(D) * 0.01).astype(np.float32),
    )
    print("loss:", out)
